# revision 1
# baseline (speedup 1.0000x reference)
"""Distributed Trainium2 kernel for nn_ACSConv (Chebyshev graph conv over a
block-Laplacian, K=8 terms, 2.56M-nnz SpMM x7 + dense feature matmuls).

Sharding: core c owns block-Laplacian rows [c*20000, (c+1)*20000) -- exactly
angle-block c, so tile(x,8)'s shard is x itself and feat()'s column block c
lives wholly on core c.  Each SpMM gathers neighbor features from a replicated
(bf16, zero-padded-to-256B-rows) table via dma_gather, segment-sums them into
PSUM with one-hot matmuls (one fused DVE tensor_scalar builds each one-hot),
and AllGathers the new shard into the next table.  Dense output matmuls read
transposed T_k windows in a late phase; the host sums per-core partials.
"""

import sys

import numpy as np

sys.path.insert(0, "/opt/trn_rl_repo")

# ---------------- problem constants (hardcoded per spec) ----------------
N = 20000          # nodes
F = 64             # in/out feature size
KCH = 8            # Chebyshev terms
ANG = 8            # angles
NCORE = 8
M = ANG * N        # block-Laplacian dim
SHARD = M // NCORE  # rows per core = 20000
SUBAG = 8          # sub-allgather groups
SUBROWS = SHARD // SUBAG   # 2500
NBLK = 5           # gather table blocks (32000 <= int16 max)
BLKSZ = M // NBLK  # 32000
WROWS = 128        # rows per window (PSUM partition capacity)
WPS = 8            # windows per super-window (one PSUM bank)
NWIN_REAL = (SHARD + WROWS - 1) // WROWS   # 157
NSW = (NWIN_REAL + WPS - 1) // WPS          # 20
NWIN = NSW * WPS                            # 160 (3 phantom)
ROWPAD = NWIN * WROWS                       # 20480


def _bf16(a):
    import ml_dtypes
    return np.asarray(a, dtype=np.float32).astype(ml_dtypes.bfloat16)


def _table_pos(cols):
    """Map global angle-major column index -> table position (sub-AG friendly
    layout: pos = j*SHARD + core*SUBROWS + r)."""
    c = cols // SHARD
    loc = cols % SHARD
    j = loc // SUBROWS
    r = loc % SUBROWS
    return j * SHARD + c * SUBROWS + r


def preprocess(x, ls_vals, weight, ls_rows, ls_cols):
    """Build per-core input maps + compile-time K_c."""
    import ml_dtypes
    bf = ml_dtypes.bfloat16

    pos = _table_pos(ls_cols.astype(np.int64))
    blk = pos // BLKSZ
    bidx = (pos % BLKSZ).astype(np.int16)
    core = ls_rows // SHARD
    lrow_all = ls_rows % SHARD
    win = lrow_all // WROWS
    wrow = (lrow_all % WROWS).astype(np.int32)
    sw = win // WPS
    wp = win % WPS

    # group key: (core, sw, blk, wp)
    key = ((core.astype(np.int64) * NSW + sw) * NBLK + blk) * WPS + wp
    order = np.argsort(key, kind="stable")
    ksorted = key[order]
    # position within group
    NGROUP = NCORE * NSW * NBLK * WPS
    counts = np.bincount(ksorted, minlength=NGROUP)
    starts = np.zeros(NGROUP + 1, dtype=np.int64)
    np.cumsum(counts, out=starts[1:])
    pos_in_group = np.arange(len(order), dtype=np.int64) - starts[ksorted]

    K_c = max(4, int(-(-counts.max() // 128)))
    CPG = WPS * K_c            # chunks per gather
    GIDX = CPG * 128           # idxs per gather
    GI16 = GIDX // 16
    NG = NSW * NBLK            # gathers per spmm
    NCHUNK = NG * CPG          # chunks per spmm

    cc = pos_in_group // 128
    lane = pos_in_group % 128

    g_of = (sw[order] * NBLK + blk[order])            # gather id within core
    i_in_g = (wp[order] * K_c + cc) * 128 + lane      # idx slot within gather
    # wp-major chunk layout (one PSUM accumulation group open at a time)
    ch = sw[order] * (NBLK * CPG) + (wp[order] * NBLK + blk[order]) * K_c + cc
    core_s = core[order]

    in_maps = []
    xbf = _bf16(x)
    x_pad = np.zeros((SHARD, 2 * F), dtype=bf)
    x_pad[:, :F] = xbf

    # xT in T^T layout [NSW*128, 512]: window w=(sw*8+wp): part 64*(wp%2)+i,
    # free 128*(wp//2)+r  = x[w*128+r, i]
    # xT layout [NSW*64, WPS*128]: xT[sw*64 + i, wp*128 + r] = x[(sw*8+wp)*128+r, i]
    xr = np.zeros((ROWPAD, F), dtype=np.float32)
    xr[:N] = np.asarray(x, dtype=np.float32)
    xw = xr.reshape(NSW, WPS, WROWS, F)  # [sw, wp, r, i]
    xT = _bf16(np.transpose(xw, (0, 3, 1, 2)).reshape(NSW * F, WPS * 128))

    iota = np.arange(128, dtype=np.float32)
    consts = np.zeros((128, 256), dtype=bf)
    consts[:, :128] = _bf16(np.broadcast_to(iota, (128, 128)))
    consts[:, 128:] = _bf16(np.eye(128, dtype=np.float32))

    for c in range(NCORE):
        m = core_s == c
        gp = np.zeros((128, NCHUNK), dtype=np.int32)
        lr = np.full((128, NCHUNK), -1.0, dtype=np.float32)
        v1 = np.zeros((128, NCHUNK), dtype=np.float32)
        gp[lane[m], ch[m]] = pos[order][m]
        lr[lane[m], ch[m]] = wrow[order][m]
        v1[lane[m], ch[m]] = ls_vals[order][m]

        wc = np.ascontiguousarray(
            np.transpose(weight[:, c * F:(c + 1) * F, :], (1, 0, 2)).reshape(F, KCH * F)
        )  # [i, k*64+o]
        wc = np.concatenate([wc, wc], axis=0)  # rows 64:128 duplicate (base-partition 64 lhsT)

        in_maps.append({
            "x_pad": x_pad,
            "xT": xT,
            "Wc": _bf16(wc),
            "consts": consts,
            "gpos": gp,
            "lrow": lr.astype(np.float32),
            "vals1": v1.astype(np.float32),
            "vals2": (2.0 * v1).astype(np.float32),
        })
    return in_maps, K_c


class _staticrange:
    def __init__(self, n): self.n = n
    def __enter__(self): return range(self.n)
    def __exit__(self, *a): return False


def build(K_c, probe=0):
    import os
    import concourse.bass as bass
    import concourse.mybir as mybir
    from concourse import tile, bacc

    dt = mybir.dt
    CPG = WPS * K_c
    GIDX = CPG * 128
    GI16 = GIDX // 16
    NG = NSW * NBLK
    NCHUNK = NG * CPG

    nc = bacc.Bacc("TRN2", target_bir_lowering=False, debug=False, num_devices=NCORE)

    x_pad = nc.dram_tensor("x_pad", [SHARD, 2 * F], dt.bfloat16, kind="ExternalInput")
    xT = nc.dram_tensor("xT", [NSW * F, WPS * 128], dt.bfloat16, kind="ExternalInput")
    Wc = nc.dram_tensor("Wc", [2 * F, KCH * F], dt.bfloat16, kind="ExternalInput")
    consts = nc.dram_tensor("consts", [128, 256], dt.bfloat16, kind="ExternalInput")
    gpos = nc.dram_tensor("gpos", [128, NCHUNK], dt.int32, kind="ExternalInput")
    lrow = nc.dram_tensor("lrow", [128, NCHUNK], dt.float32, kind="ExternalInput")
    vals1 = nc.dram_tensor("vals1", [128, NCHUNK], dt.float32, kind="ExternalInput")
    vals2 = nc.dram_tensor("vals2", [128, NCHUNK], dt.float32, kind="ExternalInput")
    outT = nc.dram_tensor("outT", [F, ROWPAD], dt.float32, kind="ExternalOutput")

    tableA = nc.dram_tensor("tableA", [M, 2 * F], dt.bfloat16, kind="Internal")
    tableB = nc.dram_tensor("tableB", [M, 2 * F], dt.bfloat16, kind="Internal")
    bounce = nc.dram_tensor("bounce", [ROWPAD, 2 * F], dt.bfloat16, kind="Internal")
    TT = [xT] + [
        nc.dram_tensor(f"tt{k}", [NSW * F, WPS * 128], dt.bfloat16, kind="Internal")
        for k in range(1, KCH)
    ]

    groups = [list(range(NCORE))]

    with tile.TileContext(nc) as tc:
        with (
            tc.tile_pool(name="persist", bufs=1) as persist,
            tc.tile_pool(name="gring", bufs=8) as gring,
            tc.tile_pool(name="sring", bufs=4) as sring,
            tc.tile_pool(name="work", bufs=2) as work,
            tc.tile_pool(name="psum", bufs=2, space="PSUM") as psum_pool,
            tc.tile_pool(name="psumT", bufs=2, space="PSUM") as psumT_pool,
            tc.tile_pool(name="psumD", bufs=1, space="PSUM") as psumD_pool,
        ):
            # ---- load persistent SBUF state ----
            gpos_sb = persist.tile([128, NCHUNK], dt.int32)
            nc.sync.dma_start(gpos_sb[:], gpos[:])
            lrow_sb = persist.tile([128, NCHUNK], dt.float32)
            v1_sb = persist.tile([128, NCHUNK], dt.float32)
            v2_sb = persist.tile([128, NCHUNK], dt.float32)
            consts_sb = persist.tile([128, 256], dt.bfloat16)
            w_sb = persist.tile([2 * F, KCH * F], dt.bfloat16)
            nc.sync.dma_start(lrow_sb[:], lrow[:])
            nc.sync.dma_start(v1_sb[:], vals1[:])
            nc.sync.dma_start(v2_sb[:], vals2[:])
            nc.sync.dma_start(consts_sb[:], consts[:])
            nc.sync.dma_start(w_sb[:], Wc[:])
            iota_ap = consts_sb[:, 0:128]
            ident_at = lambda h: consts_sb[h:h + F, 128 + h:128 + h + F]

            # ---- T0 table: x_pad -> bounce -> sub-AG -> tableA ----
            nc.sync.dma_start(bounce[0:SHARD, :], x_pad[:])
            for j in range(SUBAG) if probe not in (11,) else []:
                nc.gpsimd.collective_compute(
                    "AllGather", mybir.AluOpType.bypass, replica_groups=groups,
                    ins=[bounce[j * SUBROWS:(j + 1) * SUBROWS, :].opt()],
                    outs=[tableA[j * SHARD:(j + 1) * SHARD, :].opt()],
                )

            # ---- 7 SpMM phases ----
            for k in range(1, KCH) if probe not in (1, 12, 13, 14, 15) else []:
                src = tableA if (k - 1) % 2 == 0 else tableB
                dst = tableA if k % 2 == 0 else tableB
                vsb = v1_sb if k == 1 else v2_sb
                with tc.For_i(0, NSW, 1, staggered_reset=True,
                              hint_engines=(mybir.EngineType.PE,
                                            mybir.EngineType.DVE,
                                            mybir.EngineType.Pool)) as sw:
                    ps = [psum_pool.tile([F, 512], dt.float32, tag=f"ps{i}",
                                         name=f"ps{i}") for i in range(2)]
                    gpst = work.tile([128, NBLK * CPG], dt.int32, tag="gpst")
                    nc.sync.dma_start(
                        gpst[:], gpos_sb[:, bass.ds(sw * (NBLK * CPG), NBLK * CPG)])
                    for wpi in range(WPS):
                        fq = 128 * (wpi % 4)
                        for b in range(NBLK):
                            for ci in range(K_c):
                                choff = (wpi * NBLK + b) * K_c + ci
                                g = gring.tile([128, 2 * F], dt.bfloat16, tag="g")
                                nc.gpsimd.indirect_dma_start(
                                    out=g[:],
                                    out_offset=None,
                                    in_=src[:],
                                    in_offset=bass.IndirectOffsetOnAxis(
                                        ap=gpst[:, choff:choff + 1],
                                        axis=0,
                                    ),
                                )
                                s = sring.tile([128, 128], dt.bfloat16)
                                nc.vector.tensor_scalar(
                                    s[:], iota_ap,
                                    lrow_sb[:, bass.ds(sw * (NBLK * CPG) + choff, 1)],
                                    vsb[:, bass.ds(sw * (NBLK * CPG) + choff, 1)],
                                    mybir.AluOpType.is_equal, mybir.AluOpType.mult,
                                )
                                nc.tensor.matmul(
                                    ps[wpi // 4][:, fq:fq + 128],
                                    g[:, 0:F], s[:],
                                    start=(wpi % 4 == 0 and b == 0 and ci == 0),
                                    stop=(wpi % 4 == 3 and b == NBLK - 1
                                          and ci == K_c - 1),
                                )
                    # recurrence -> Tn^T (bf16) in SBUF
                    tn = work.tile([F, WPS * 128], dt.bfloat16, tag="tn")
                    if k == 1:
                        for i in range(2):
                            nc.vector.tensor_copy(tn[:, i * 512:(i + 1) * 512],
                                                  ps[i][:])
                    else:
                        tp = work.tile([F, WPS * 128], dt.bfloat16, tag="tp")
                        nc.sync.dma_start(tp[:], TT[k - 2][bass.ds(sw * F, F), :])
                        for i in range(2):
                            nc.vector.tensor_tensor(
                                out=tn[:, i * 512:(i + 1) * 512], in0=ps[i][:],
                                in1=tp[:, i * 512:(i + 1) * 512],
                                op=mybir.AluOpType.subtract,
                            )
                    nc.sync.dma_start(TT[k][bass.ds(sw * F, F), :], tn[:])
                    if k < KCH - 1:
                        # transpose windows to row-major -> bounce
                        for wpi in range(WPS):
                            pt = psumT_pool.tile([128, F], dt.bfloat16)
                            nc.tensor.transpose(
                                pt[:], tn[:, wpi * 128:wpi * 128 + 128], ident_at(0))
                            pts = work.tile([128, F], dt.bfloat16, tag="pts")
                            nc.scalar.copy(pts[:], pt[:])
                            nc.sync.dma_start(
                                bounce[bass.ds((sw * WPS + wpi) * 128, 128), 0:F],
                                pts[:])
                if k < KCH - 1 and probe != 11:
                    for j in range(SUBAG):
                        nc.gpsimd.collective_compute(
                            "AllGather", mybir.AluOpType.bypass, replica_groups=groups,
                            ins=[bounce[j * SUBROWS:(j + 1) * SUBROWS, :].opt()],
                            outs=[dst[j * SHARD:(j + 1) * SHARD, :].opt()],
                        )

            # ---- dense output phase ----
            for _ in ([1] if probe not in (12, 16) else []):
             with (tc.For_i(0, NSW, 1, staggered_reset=True,
                            hint_engines=(mybir.EngineType.PE,))
                   if probe != 15 else _staticrange(NSW)) as wbs:
              for wb in (wbs if probe == 15 else [wbs]):
                 tts = []
                 for k in range(KCH):
                     t = work.tile([F, WPS * 128], dt.bfloat16, tag=f"dtt{k}", name=f"dtt{k}")
                     nc.sync.dma_start(t[:], TT[k][bass.ds(wb * F, F), :])
                     tts.append(t)
                 pd = [psumD_pool.tile([F, 512], dt.float32, tag=f"pd{i}",
                                       name=f"pd{i}") for i in range(2)]
                 for wpi in range(WPS):
                     for k in range(KCH):
                         nc.tensor.matmul(
                             pd[wpi // 4][:, (wpi % 4) * 128:(wpi % 4) * 128 + 128],
                             w_sb[0:F, k * F:(k + 1) * F],
                             tts[k][:, wpi * 128:wpi * 128 + 128],
                             start=(wpi % 4 == 0 and k == 0),
                             stop=(wpi % 4 == 3 and k == KCH - 1),
                         )
                 for i in range(2):
                     pdc = work.tile([F, 512], dt.float32, tag=f"pdc{i}")
                     nc.scalar.copy(pdc[:], pd[i][:])
                     nc.sync.dma_start(
                         outT[:, bass.ds(wb * 1024 + 512 * i, 512)], pdc[:])

    nc.finalize()
    return nc


def kernel(x, ls_vals, weight, bias, ls_rows, ls_cols):
    from concourse.bass_utils import run_bass_kernel_spmd

    in_maps, K_c = preprocess(x, ls_vals, weight, ls_rows, ls_cols)
    nc = build(K_c)
    res = run_bass_kernel_spmd(nc, in_maps, core_ids=list(range(NCORE)))
    out = np.zeros((F, N), dtype=np.float32)
    for c in range(NCORE):
        out += np.asarray(res.results[c]["outT"], dtype=np.float32)[:, :N]
    return (out.T + np.asarray(bias, dtype=np.float32)[None, :]).astype(np.float32)

